# revision 27
# baseline (speedup 1.0000x reference)
"""Trainium2 Bass kernel for nn_Conv2d_shared_res_bit_baens.

Computes, for inputs x:(48,128,64,64) U:(6,147456) bp:(7,147456) u:(7,147456):
  - gumbel-sigmoid gates from (u, bp)
  - 2-level residual STE quantization of U  (round-to-nearest-even)
  - per-member 3x3 conv weights w[n] = v1*g1 + v2*g2*(g1>0)
  - grouped conv: out[b] = conv2d(x[b], W[b % 6], pad=1)

Sharding: data-parallel over batch across 8 NeuronCores (6 images each, one
per ensemble member); the weight-construction pipeline is replicated on every
core.  The conv itself is 9 shift-matmuls (128x128 per tap) accumulated in
PSUM over a 66-wide zero-padded spatial layout.
"""

import os
from contextlib import ExitStack

import numpy as np

import concourse.bass as bass
import concourse.mybir as mybir
import concourse.tile as tile
from concourse.bass_utils import run_bass_kernel_spmd

F32 = mybir.dt.float32
F32R = mybir.dt.float32r
I8 = mybir.dt.int8
AF = mybir.ActivationFunctionType
ALU = mybir.AluOpType

N_CORES = 8
N = 6                      # ensemble members
C = 128                    # in/out channels
D = 147456                 # flattened weight size per member
FD = D // C                # 1152 free-dim elements per partition (natural layout)
KQ = 9                     # 3x3 taps
H = W = 64
PW = 66                    # padded spatial width
GRID = PW * PW             # 4356
XW = 4360                  # x_pad row: [1 guard][4356 grid][3 tail guard]
OUT0 = PW                  # first output flat position (row y=1, x=0)
OUTW = 4224                # 64 rows * 66 cols
MAGIC = float(np.float32(12582912.0))   # 1.5 * 2**23 (RNE integer rounding)
ZP1, ZP2 = 1.2, -0.2
# conv output chunking: all chunks >=256 wide so fp32r matmuls run at 1 cyc/row
CHUNKS = [(0, 512), (512, 512), (1024, 512), (1536, 512), (2048, 512),
          (2560, 512), (3072, 512), (3584, 320), (3904, 320)]
assert sum(w for _, w in CHUNKS) == OUTW


# ---------------------------------------------------------------------------
# host-side scalar prep (bit-exact mirrors of the fp32 scalar ops in the ref)
# ---------------------------------------------------------------------------

def _round_f32(x):
    return np.round(x.astype(np.float32)).astype(np.float32)


def _find_threshold(s1, k):
    """Smallest f32 v with round(fl(v / s1)) >= k  (s1 > 0, RNE round)."""
    pred = lambda v: float(np.round(np.float32(v) / s1)) >= k
    v = np.float32(s1 * np.float32(k - 0.5))
    step = np.float32(max(abs(float(v)), 1.0) * 1e-5)
    lo, hi = v, v
    while pred(lo):
        lo = np.float32(lo - step)
    while not pred(hi):
        hi = np.float32(hi + step)
    # bisect on the f32 bit lattice
    def bits(f):
        i = np.float32(f).view(np.int32).item()
        return i if i >= 0 else (0x80000000 - i)  # monotone map

    def unbits(i):
        i = i if i >= 0 else -(i - 0x80000000)
        if i >= 0x80000000:
            i = 0x80000000 - i
        return np.int32(i).view(np.float32)

    blo, bhi = bits(lo), bits(hi)
    while bhi - blo > 1:
        mid = (blo + bhi) // 2
        if pred(unbits(mid)):
            bhi = mid
        else:
            blo = mid
    return unbits(bhi)


def _host_scalars(U):
    beta = np.float32(U.max())
    alpha = np.float32(U.min())
    s1 = np.float32((beta - alpha) / np.float32(3.0))
    s2 = np.float32(s1 / np.float32(5.0))
    t = (U / s1).astype(np.float32)
    qr = _round_f32(t)
    qmin, qmax = int(qr.min()), int(qr.max())
    thr = [float(_find_threshold(s1, k)) for k in range(qmin + 1, qmax + 1)]
    # verify the comparison form reproduces round(U/s1) exactly
    qdev = np.full(U.shape, qmin, np.float32)
    for T in thr:
        qdev += (U >= np.float32(T)).astype(np.float32)
    assert np.array_equal(qdev, qr), "q1 threshold construction failed"
    # verify the device affine acc*s1 + (qmin*s1) hits fl(s1*q) for every q
    qmin_s1 = np.float32(np.float32(qmin) * s1)
    for q in range(qmin, qmax + 1):
        acc = np.float32(q - qmin)
        dev = np.float32(np.float32(acc * s1) + qmin_s1)
        ref = np.float32(s1 * np.float32(q))
        assert dev == ref, f"v1 affine inexact for q={q}"
    r2hi = np.float32(np.float64(1.0) / np.float64(s2))
    r2lo = np.float32(np.float64(1.0) / np.float64(s2) - np.float64(r2hi))
    sc_gate = np.float32(np.float32(ZP1) - np.float32(ZP2))
    return dict(
        s1=float(s1), s2=float(s2), qmin=qmin, thr=tuple(thr),
        qmin_s1=float(qmin_s1), r2hi=float(r2hi), r2lo=float(r2lo),
        sc_gate=float(sc_gate), zp2=float(np.float32(ZP2)),
        r6hi=float(np.float32(np.float64(1.0) / 6.0)),
        r6lo=float(np.float32(np.float64(1.0) / 6.0 - np.float64(np.float32(np.float64(1.0) / 6.0)))),
    )


# ---------------------------------------------------------------------------
# device program
# ---------------------------------------------------------------------------

_NO_SPLIT_TYPES = ("InstNoOp", "InstEventSemaphore",
                   "InstBranch", "InstSemaphoreOp", "InstRegister")


def _split_excess_dma_waits(nc, max_waits=1):
    """walrus's pseudo-instruction formats support very few sync waits per
    instruction, but Tile can emit 3+ on pool-slot reuse (reader WAR +
    prior-DMA WAW + own lane).  Move the excess onto a NoOp on the issuing
    engine right before the instruction — the sequencer executes its waits in
    order, so semantics are unchanged."""
    import bass_rust

    func = nc.m.functions[0]
    for blk in func.blocks:
        insts = list(blk.instructions)
        new_insts = []
        changed = False
        for ins in insts:
            si = ins.sync_info
            tname = type(ins).__name__
            if (si is not None and len(si.on_wait) > max_waits
                    and not any(tname.startswith(p) for p in _NO_SPLIT_TYPES)):
                waits = list(si.on_wait)
                moved, kept = waits[:-max_waits], waits[-max_waits:]
                for j, w in enumerate(moved):
                    nop = mybir.InstNoOp(
                        name=f"{ins.name}-waitnop{j}", ins=[], outs=[])
                    nop.engine = ins.engine
                    nop.sync_info = bass_rust.SyncInfo(on_wait=[w],
                                                       on_update=[])
                    new_insts.append(nop)
                ins.sync_info = bass_rust.SyncInfo(
                    on_wait=kept, on_update=list(si.on_update))
                changed = True
            new_insts.append(ins)
        if changed:
            blk.instructions = new_insts


def _build_program(sc):
    nc = bass.Bass(target_bir_lowering=False, debug=False, trn_type="TRN2")

    xp_t = nc.dram_tensor("xp", [N, C, XW], F32, kind="ExternalInput")
    U_t = nc.dram_tensor("U", [N, D], F32, kind="ExternalInput")
    bp_t = nc.dram_tensor("bp", [N + 1, D], F32, kind="ExternalInput")
    u_t = nc.dram_tensor("u", [N + 1, D], F32, kind="ExternalInput")
    id_t = nc.dram_tensor("ident", [C, C], F32, kind="ExternalInput")
    out_t = nc.dram_tensor("out", [N, C, OUTW], F32, kind="ExternalOutput")

    s1, s2 = sc["s1"], sc["s2"]

    with tile.TileContext(nc) as tc, ExitStack() as ctx:
        cpool = ctx.enter_context(tc.tile_pool(name="const", bufs=1))
        ident = cpool.tile([C, C], F32)
        nc.sync.dma_start(ident[:], id_t[:])

        # resident across the whole kernel
        q1p = ctx.enter_context(tc.tile_pool(name="q1", bufs=N))
        q2p = ctx.enter_context(tc.tile_pool(name="q2", bufs=N))
        ps_tp = ctx.enter_context(tc.tile_pool(name="pstp", bufs=2, space="PSUM"))
        ps_cv = ctx.enter_context(tc.tile_pool(name="pscv", bufs=4, space="PSUM"))

        q1_i8 = [q1p.tile([C, FD], I8, tag="q1", name=f"q1_{n}") for n in range(N)]
        q2_i8 = [q2p.tile([C, FD], I8, tag="q2", name=f"q2_{n}") for n in range(N)]

        # ---------------- phase 1: quantization (exact round1) -------------
        # Scoped pools: released before the conv-phase pools are allocated so
        # the same SBUF space is reused.
        with tc.tile_pool(name="ph1", bufs=N) as p1, \
             tc.tile_pool(name="ph1c", bufs=2) as p1c, \
             tc.tile_pool(name="ph1t", bufs=4) as p1t:
            Usb = []
            for n in range(N):
                t = p1.tile([C, FD], F32, tag="U", name=f"U_{n}")
                nc.sync.dma_start(t[:], U_t[n].rearrange("(p f) -> p f", p=C))
                Usb.append(t)
            v1 = []
            for n in range(N):
                acc = p1c.tile([C, FD], F32, tag="acc", name=f"acc_{n}")
                thr = sc["thr"]
                nc.vector.tensor_scalar(acc[:], Usb[n][:], float(thr[0]), None,
                                        ALU.is_ge)
                for j, T in enumerate(thr[1:]):
                    nxt = p1c.tile([C, FD], F32, tag="acc", name=f"acc_{n}_{j}")
                    nc.vector.scalar_tensor_tensor(nxt[:], Usb[n][:], float(T),
                                                   acc[:], ALU.is_ge, ALU.add)
                    acc = nxt
                # int8 copy of acc (q1 - qmin), used later to rebuild v1
                nc.vector.tensor_copy(q1_i8[n][:], acc[:])
                v1t = p1.tile([C, FD], F32, tag="v1", name=f"v1_{n}")
                nc.vector.tensor_scalar(v1t[:], acc[:], s1, sc["qmin_s1"],
                                        ALU.mult, ALU.add)
                v1.append(v1t)
            # m = mean over members: neuron-XLA jnp.mean == sequential sum,
            # then a single fl(* 1/6)
            msum = v1[0]
            for n in range(1, N):
                nxt = p1c.tile([C, FD], F32, tag="msum", name=f"msum_{n}")
                nc.gpsimd.tensor_tensor(nxt[:], msum[:], v1[n][:], ALU.add)
                msum = nxt
            m = p1c.tile([C, FD], F32, tag="msum", name="m")
            nc.scalar.mul(m[:], msum[:], sc["r6hi"])
            for n in range(N):
                d = p1t.tile([C, FD], F32, tag="p1t", name=f"d_{n}")
                nc.gpsimd.tensor_tensor(d[:], Usb[n][:], m[:], ALU.subtract)
                phi = p1t.tile([C, FD], F32, tag="p1t", name=f"phi_{n}")
                nc.vector.tensor_scalar(phi[:], d[:], sc["r2hi"], None, ALU.mult)
                t2 = p1t.tile([C, FD], F32, tag="p1t", name=f"t2_{n}")
                nc.vector.scalar_tensor_tensor(t2[:], d[:], sc["r2lo"], phi[:],
                                               ALU.mult, ALU.add)
                t2m = p1t.tile([C, FD], F32, tag="p1t", name=f"t2m_{n}")
                nc.scalar.activation(t2m[:], t2[:], AF.Copy, bias=MAGIC)
                q2f = p1t.tile([C, FD], F32, tag="p1t", name=f"q2f_{n}")
                nc.scalar.activation(q2f[:], t2m[:], AF.Copy, bias=-MAGIC)
                nc.vector.tensor_copy(q2_i8[n][:], q2f[:])

        # ---------------- conv-phase pools (reuse phase-1 space) -----------
        gshared = ctx.enter_context(tc.tile_pool(name="gshared", bufs=1))
        wtp = ctx.enter_context(tc.tile_pool(name="wt", bufs=3 * KQ))
        xpool = ctx.enter_context(tc.tile_pool(name="xpool", bufs=1))
        xrpool = ctx.enter_context(tc.tile_pool(name="xrpool", bufs=2))
        gatep = ctx.enter_context(tc.tile_pool(name="gates", bufs=3))
        wnatp = ctx.enter_context(tc.tile_pool(name="wnat", bufs=2))
        wtmp = ctx.enter_context(tc.tile_pool(name="wtmp", bufs=6))
        obufp = ctx.enter_context(tc.tile_pool(name="obuf", bufs=4))
        rowp = ctx.enter_context(tc.tile_pool(name="rows", bufs=2))
        gtmp = ctx.enter_context(tc.tile_pool(name="gtmp", bufs=6))

        # ---------------- gates ------------------------------------------
        def gate_row(r, dst_tile):
            usb = rowp.tile([C, FD], F32, tag="u", name=f"u_{r}")
            nc.gpsimd.dma_start(usb[:], u_t[r].rearrange("(p f) -> p f", p=C))
            bpsb = rowp.tile([C, FD], F32, tag="bp", name=f"bp_{r}")
            nc.gpsimd.dma_start(bpsb[:], bp_t[r].rearrange("(p f) -> p f", p=C))
            lnu = gtmp.tile([C, FD], F32, tag="gt", name=f"lnu_{r}")
            nc.scalar.activation(lnu[:], usb[:], AF.Ln)
            # on ACT (not DVE) so the u-row tile has single-engine readers:
            # its pool-slot-reuse DMA then needs only one WAR wait
            om = gtmp.tile([C, FD], F32, tag="gt", name=f"om_{r}")
            nc.scalar.activation(om[:], usb[:], AF.Copy, bias=1.0, scale=-1.0)
            ln1m = gtmp.tile([C, FD], F32, tag="gt", name=f"ln1m_{r}")
            nc.scalar.activation(ln1m[:], om[:], AF.Ln)
            gg = gtmp.tile([C, FD], F32, tag="gt", name=f"gg_{r}")
            nc.gpsimd.tensor_tensor(gg[:], lnu[:], ln1m[:], ALU.subtract)
            ge = gtmp.tile([C, FD], F32, tag="gt", name=f"ge_{r}")
            nc.gpsimd.tensor_tensor(ge[:], gg[:], bpsb[:], ALU.add)
            sg = gtmp.tile([C, FD], F32, tag="gt", name=f"sg_{r}")
            nc.scalar.activation(sg[:], ge[:], AF.Sigmoid)
            gr = gtmp.tile([C, FD], F32, tag="gt", name=f"gr_{r}")
            nc.vector.tensor_scalar(gr[:], sg[:], sc["sc_gate"], sc["zp2"],
                                    ALU.mult, ALU.add)
            nc.vector.tensor_scalar(dst_tile[:], gr[:], 0.0, 1.0,
                                    ALU.max, ALU.min)

        g2 = gshared.tile([C, FD], F32, tag="g2", name="g2")
        gate_row(N, g2)

        # ---------------- per-member: weights + transposes + conv ---------
        wt = [[wtp.tile([C, C], F32R, tag="wt", name=f"wt_{n}_{q}")
               for q in range(KQ)] for n in range(N)]

        for n in range(N):
            xsb = xpool.tile([C, XW], F32, tag="x", name=f"x_{n}")
            nc.gpsimd.dma_start(xsb[:], xp_t[n])
            # fp32r matmul operands must be explicitly rounded by a compute op
            xr = xrpool.tile([C, XW], F32R, tag="xr", name=f"xr_{n}")
            nc.vector.tensor_copy(xr[:], xsb[:])

            g1 = gatep.tile([C, FD], F32, tag="g1", name=f"g1_{n}")
            gate_row(n, g1)

            v1r = wtmp.tile([C, FD], F32, tag="wtmp", name=f"v1r_{n}")
            nc.scalar.activation(v1r[:], q1_i8[n][:], AF.Copy,
                                 bias=sc["qmin_s1"], scale=s1)
            v2r = wtmp.tile([C, FD], F32, tag="wtmp", name=f"v2r_{n}")
            nc.scalar.activation(v2r[:], q2_i8[n][:], AF.Copy,
                                 bias=0.0, scale=s2)
            bpre = wtmp.tile([C, FD], F32, tag="wtmp", name=f"bpre_{n}")
            nc.gpsimd.tensor_tensor(bpre[:], v2r[:], g2[:], ALU.mult)
            bsel = wtmp.tile([C, FD], F32, tag="wtmp", name=f"bsel_{n}")
            nc.vector.scalar_tensor_tensor(bsel[:], g1[:], 0.0, bpre[:],
                                           ALU.is_gt, ALU.mult)
            cc = wtmp.tile([C, FD], F32, tag="wtmp", name=f"cc_{n}")
            nc.gpsimd.tensor_tensor(cc[:], v1r[:], g1[:], ALU.mult)
            wn = wnatp.tile([C, FD], F32, tag="wnat", name=f"wn_{n}")
            nc.vector.tensor_tensor(wn[:], bsel[:], cc[:], ALU.add)

            wq = wn[:].rearrange("p (i q) -> p q i", q=KQ)
            for q in range(KQ):
                tp = ps_tp.tile([C, C], F32, tag="tp", name=f"tp_{n}_{q}")
                nc.tensor.transpose(tp[:], wq[:, q, :], ident[:])
                nc.any.tensor_copy(wt[n][q][:], tp[:])

            for off, cw in CHUNKS:
                ps = ps_cv.tile([C, 512], F32, tag="cv", name=f"cv_{n}_{off}")
                for ky in range(3):
                    for kx in range(3):
                        q = ky * 3 + kx
                        a = 1 + OUT0 + off + (ky - 1) * PW + (kx - 1)
                        nc.tensor.matmul(
                            ps[:, :cw],
                            wt[n][q][:],
                            xr[:, a:a + cw],
                            start=(q == 0), stop=(q == KQ - 1),
                        )
                ob = obufp.tile([C, 512], F32, tag="ob", name=f"ob_{n}_{off}")
                nc.any.tensor_copy(ob[:, :cw], ps[:, :cw])
                nc.scalar.dma_start(out_t[n][:, off:off + cw], ob[:, :cw])

    _split_excess_dma_waits(nc)
    return nc


# ---------------------------------------------------------------------------
# entry point
# ---------------------------------------------------------------------------

_prog_cache = {}
last_results = None  # BassKernelResults of the most recent kernel() call

_AXON_SO = "/opt/axon/libaxon_pjrt.so"


def _build_ntff_hook():
    """(output_dir, device_ids) -> contextmanager driving NRT profiling via
    the axon PJRT .so — the slim-container equivalent of axon.trn.ntff_profile."""
    import contextlib
    import ctypes
    import sys as _sys

    if not os.path.exists(_AXON_SO):
        return None
    lib = ctypes.CDLL(_AXON_SO)
    if not hasattr(lib, "axon_start_nrt_profile"):
        return None
    lib.axon_start_nrt_profile.argtypes = [ctypes.POINTER(ctypes.c_int64),
                                           ctypes.c_size_t]
    lib.axon_start_nrt_profile.restype = ctypes.c_int64
    lib.axon_stop_nrt_profile.argtypes = [ctypes.c_char_p]
    lib.axon_stop_nrt_profile.restype = ctypes.c_int64

    @contextlib.contextmanager
    def _hook(output_dir, device_ids):
        import jax
        jax.devices()
        if device_ids:
            ids = (ctypes.c_int64 * len(device_ids))(*device_ids)
            rc = lib.axon_start_nrt_profile(ids, len(device_ids))
        else:
            rc = lib.axon_start_nrt_profile(None, 0)
        if rc != 0:
            raise RuntimeError(f"axon_start_nrt_profile rc={rc}")
        try:
            yield
        finally:
            n = lib.axon_stop_nrt_profile(str(output_dir).encode())
            print(f"profile: {n} file(s) written to {output_dir}",
                  file=_sys.stderr)

    return _hook


def _ensure_ntff_hook():
    """Make `antenv.axon_hooks.get_axon_ntff_profile_hook` importable so the
    BASS_TRACE path in run_bass_kernel_spmd works (or degrades gracefully)."""
    import sys as _sys
    import types

    try:
        from antenv.axon_hooks import get_axon_ntff_profile_hook  # noqa: F401
        return
    except ImportError:
        pass
    mod = types.ModuleType("antenv.axon_hooks")
    state = {}

    def get_axon_ntff_profile_hook():
        if "h" not in state:
            try:
                state["h"] = _build_ntff_hook()
            except Exception:
                state["h"] = None
        return state["h"]

    mod.get_axon_ntff_profile_hook = get_axon_ntff_profile_hook
    _sys.modules["antenv.axon_hooks"] = mod
    try:
        import antenv
        antenv.axon_hooks = mod
    except ImportError:
        pass


def _get_program(sc):
    key = (sc["s1"], sc["s2"], sc["qmin"], sc["thr"])
    if key not in _prog_cache:
        _prog_cache[key] = _build_program(sc)
    return _prog_cache[key]


def kernel(x, U, bp, u):
    x = np.ascontiguousarray(x, dtype=np.float32)
    U = np.ascontiguousarray(U, dtype=np.float32)
    bp = np.ascontiguousarray(bp, dtype=np.float32)
    u = np.ascontiguousarray(u, dtype=np.float32)
    B = x.shape[0]
    assert B == N_CORES * N and x.shape[1] == C

    sc = _host_scalars(U)
    # q2 must fit int8 (device stores round((U-m)/s2) as int8)
    s1f, s2f = np.float32(sc["s1"]), np.float32(sc["s2"])
    v1h = (s1f * _round_f32(U / s1f)).astype(np.float32)
    mseq = v1h[0]
    for i in range(1, N):
        mseq = (mseq + v1h[i]).astype(np.float32)
    mh = (mseq * np.float32(sc["r6hi"])).astype(np.float32)
    q2h = _round_f32((U - mh).astype(np.float32) / s2f)
    assert np.abs(q2h).max() <= 126, "q2 exceeds int8 range"
    nc = _get_program(sc)

    # zero-padded 66-wide spatial layout, one guard element in front
    grid = np.zeros((B, C, PW, PW), np.float32)
    grid[:, :, 1:H + 1, 1:W + 1] = x
    xp = np.zeros((B, C, XW), np.float32)
    xp[:, :, 1:1 + GRID] = grid.reshape(B, C, GRID)
    xp = xp.reshape(N_CORES, N, C, XW)

    ident = np.eye(C, dtype=np.float32)
    in_maps = [
        {"xp": xp[c], "U": U, "bp": bp, "u": u, "ident": ident}
        for c in range(N_CORES)
    ]
    _ensure_ntff_hook()
    global last_results
    last_results = run_bass_kernel_spmd(nc, in_maps, list(range(N_CORES)))
    res = last_results.results

    out = np.empty((B, C, H, W), np.float32)
    for c in range(N_CORES):
        o = res[c]["out"].reshape(N, C, H, PW)
        out[c * N:(c + 1) * N] = o[:, :, :, 1:W + 1]
    return out


# revision 31
# speedup vs baseline: 1.1502x; 1.1502x over previous
"""Trainium2 Bass kernel for nn_Conv2d_shared_res_bit_baens.

Computes, for inputs x:(48,128,64,64) U:(6,147456) bp:(7,147456) u:(7,147456):
  - gumbel-sigmoid gates from (u, bp)
  - 2-level residual STE quantization of U  (round-to-nearest-even)
  - per-member 3x3 conv weights w[n] = v1*g1 + v2*g2*(g1>0)
  - grouped conv: out[b] = conv2d(x[b], W[b % 6], pad=1)

Sharding: the conv is data-parallel over batch across 8 NeuronCores (6 images
per core, one per ensemble member).  The weight-construction pipeline is
sharded 8-ways along the input-channel axis (16 in-channels per core — 1/8 of
the elementwise work each) and the finished weights are AllGathered, after
which every core runs its own 6 images.  The conv itself is 9 shift-matmuls
(128x128 per tap, fp32r) accumulated in PSUM over a 66-wide zero-padded
spatial layout.
"""

import os
from contextlib import ExitStack

import numpy as np

import concourse.bass as bass
import concourse.mybir as mybir
import concourse.tile as tile
from concourse.bass_utils import run_bass_kernel_spmd

F32 = mybir.dt.float32
F32R = mybir.dt.float32r
AF = mybir.ActivationFunctionType
ALU = mybir.AluOpType

N_CORES = 8
N = 6                      # ensemble members
C = 128                    # in/out channels
D = 147456                 # flattened weight size per member
FD = D // C                # 1152 free-dim elements per partition (natural layout)
KQ = 9                     # 3x3 taps
IB = C // N_CORES          # 16 in-channels per core (weight-pipeline shard)
FS = IB * KQ               # 144 free-dim elements per shard per member
H = W = 64
PW = 66                    # padded spatial width
GRID = PW * PW             # 4356
XW = 4360                  # x_pad row: [1 guard][4356 grid][3 tail guard]
OUT0 = PW                  # first output flat position (row y=1, x=0)
OUTW = 4224                # 64 rows * 66 cols
MAGIC = float(np.float32(12582912.0))   # 1.5 * 2**23 (RNE integer rounding)
ZP1, ZP2 = 1.2, -0.2
# conv output chunking: all chunks >=256 wide so fp32r matmuls run at 1 cyc/row
CHUNKS = [(0, 512), (512, 512), (1024, 512), (1536, 512), (2048, 512),
          (2560, 512), (3072, 512), (3584, 320), (3904, 320)]
assert sum(w for _, w in CHUNKS) == OUTW


# ---------------------------------------------------------------------------
# host-side scalar prep (bit-exact mirrors of the fp32 scalar ops in the ref)
# ---------------------------------------------------------------------------

def _round_f32(x):
    return np.round(x.astype(np.float32)).astype(np.float32)


def _find_threshold(s1, k):
    """Smallest f32 v with round(fl(v / s1)) >= k  (s1 > 0, RNE round)."""
    pred = lambda v: float(np.round(np.float32(v) / s1)) >= k
    v = np.float32(s1 * np.float32(k - 0.5))
    step = np.float32(max(abs(float(v)), 1.0) * 1e-5)
    lo, hi = v, v
    while pred(lo):
        lo = np.float32(lo - step)
    while not pred(hi):
        hi = np.float32(hi + step)
    # bisect on the f32 bit lattice
    def bits(f):
        i = np.float32(f).view(np.int32).item()
        return i if i >= 0 else (0x80000000 - i)  # monotone map

    def unbits(i):
        i = i if i >= 0 else -(i - 0x80000000)
        if i >= 0x80000000:
            i = 0x80000000 - i
        return np.int32(i).view(np.float32)

    blo, bhi = bits(lo), bits(hi)
    while bhi - blo > 1:
        mid = (blo + bhi) // 2
        if pred(unbits(mid)):
            bhi = mid
        else:
            blo = mid
    return unbits(bhi)


def _host_scalars(U):
    beta = np.float32(U.max())
    alpha = np.float32(U.min())
    s1 = np.float32((beta - alpha) / np.float32(3.0))
    s2 = np.float32(s1 / np.float32(5.0))
    t = (U / s1).astype(np.float32)
    qr = _round_f32(t)
    qmin, qmax = int(qr.min()), int(qr.max())
    thr = [float(_find_threshold(s1, k)) for k in range(qmin + 1, qmax + 1)]
    # verify the comparison form reproduces round(U/s1) exactly
    qdev = np.full(U.shape, qmin, np.float32)
    for T in thr:
        qdev += (U >= np.float32(T)).astype(np.float32)
    assert np.array_equal(qdev, qr), "q1 threshold construction failed"
    # verify the device affine acc*s1 + (qmin*s1) hits fl(s1*q) for every q
    qmin_s1 = np.float32(np.float32(qmin) * s1)
    for q in range(qmin, qmax + 1):
        acc = np.float32(q - qmin)
        dev = np.float32(np.float32(acc * s1) + qmin_s1)
        ref = np.float32(s1 * np.float32(q))
        assert dev == ref, f"v1 affine inexact for q={q}"
    r2hi = np.float32(np.float64(1.0) / np.float64(s2))
    r2lo = np.float32(np.float64(1.0) / np.float64(s2) - np.float64(r2hi))
    sc_gate = np.float32(np.float32(ZP1) - np.float32(ZP2))
    return dict(
        s1=float(s1), s2=float(s2), qmin=qmin, thr=tuple(thr),
        qmin_s1=float(qmin_s1), r2hi=float(r2hi), r2lo=float(r2lo),
        sc_gate=float(sc_gate), zp2=float(np.float32(ZP2)),
        r6hi=float(np.float32(np.float64(1.0) / 6.0)),
    )


# ---------------------------------------------------------------------------
# post-pass: walrus allows only ONE sync wait per instruction
# ---------------------------------------------------------------------------

_NO_SPLIT_TYPES = ("InstNoOp", "InstEventSemaphore",
                   "InstBranch", "InstSemaphoreOp", "InstRegister")


def _split_excess_dma_waits(nc, max_waits=1):
    """walrus's pseudo-instruction formats support a single sync wait per
    instruction, but Tile can emit several (pool-slot reuse: reader WAR +
    prior-writer WAW + own DMA lane).  Move the excess onto NoOps on the
    issuing engine right before the instruction — the sequencer executes the
    waits in order, so semantics are unchanged."""
    import bass_rust

    func = nc.m.functions[0]
    for blk in func.blocks:
        insts = list(blk.instructions)
        new_insts = []
        changed = False
        for ins in insts:
            si = ins.sync_info
            tname = type(ins).__name__
            if (si is not None and len(si.on_wait) > max_waits
                    and not any(tname.startswith(p) for p in _NO_SPLIT_TYPES)):
                waits = list(si.on_wait)
                moved, kept = waits[:-max_waits], waits[-max_waits:]
                for j, w in enumerate(moved):
                    nop = mybir.InstNoOp(
                        name=f"{ins.name}-waitnop{j}", ins=[], outs=[])
                    nop.engine = ins.engine
                    nop.sync_info = bass_rust.SyncInfo(on_wait=[w],
                                                       on_update=[])
                    new_insts.append(nop)
                ins.sync_info = bass_rust.SyncInfo(
                    on_wait=kept, on_update=list(si.on_update))
                changed = True
            new_insts.append(ins)
        if changed:
            blk.instructions = new_insts


# ---------------------------------------------------------------------------
# device program
# ---------------------------------------------------------------------------

def _build_program(sc):
    nc = bass.Bass(target_bir_lowering=False, debug=False, trn_type="TRN2",
                   num_devices=N_CORES)

    # per-core pre-sliced weight-pipeline inputs (host marshals):
    # ush[o, n*FS+f] = U[n, o*FD + core*FS + f], similarly for u/bp rows
    ush_t = nc.dram_tensor("ush", [C, N * FS], F32, kind="ExternalInput")
    uu_t = nc.dram_tensor("uu", [C, (N + 1) * FS], F32, kind="ExternalInput")
    ub_t = nc.dram_tensor("ub", [C, (N + 1) * FS], F32, kind="ExternalInput")
    xp_t = nc.dram_tensor("xp", [N, C, XW], F32, kind="ExternalInput")
    id_t = nc.dram_tensor("ident", [C, C], F32, kind="ExternalInput")
    out_t = nc.dram_tensor("out", [N, C, OUTW], F32, kind="ExternalOutput")

    # collective staging: local shard -> all-gathered full weights (rank-major)
    wstage_t = nc.dram_tensor("wstage", [N, C, FS], F32)
    wgath_t = nc.dram_tensor("wgath", [N_CORES, N, C, FS], F32,
                             addr_space="Shared")

    s1, s2 = sc["s1"], sc["s2"]
    NFS = N * FS

    with tile.TileContext(nc) as tc, ExitStack() as ctx:
        cpool = ctx.enter_context(tc.tile_pool(name="const", bufs=1))
        pipe = ctx.enter_context(tc.tile_pool(name="pipe", bufs=1))
        ptmp = ctx.enter_context(tc.tile_pool(name="ptmp", bufs=8))
        wnatp = ctx.enter_context(tc.tile_pool(name="wnat", bufs=3))
        wtp = ctx.enter_context(tc.tile_pool(name="wt", bufs=3 * KQ))
        xpool = ctx.enter_context(tc.tile_pool(name="xpool", bufs=2))
        xrpool = ctx.enter_context(tc.tile_pool(name="xrpool", bufs=2))
        obufp = ctx.enter_context(tc.tile_pool(name="obuf", bufs=6))
        ps_tp = ctx.enter_context(tc.tile_pool(name="pstp", bufs=2, space="PSUM"))
        ps_cv = ctx.enter_context(tc.tile_pool(name="pscv", bufs=6, space="PSUM"))

        ident = cpool.tile([C, C], F32)
        nc.sync.dma_start(ident[:], id_t[:])

        # input shards (single contiguous DMAs)
        ush = pipe.tile([C, NFS], F32, tag="ush", name="ush")
        nc.sync.dma_start(ush[:], ush_t[:])
        uu = pipe.tile([C, (N + 1) * FS], F32, tag="uu", name="uu")
        nc.sync.dma_start(uu[:], uu_t[:])
        ub = pipe.tile([C, (N + 1) * FS], F32, tag="ub", name="ub")
        nc.sync.dma_start(ub[:], ub_t[:])

        # prefetch image 0 (sync ring, right behind the pipeline inputs)
        xsb0 = xpool.tile([C, XW], F32, tag="x", name="x_0")
        nc.sync.dma_start(xsb0[:], xp_t[0])

        def pt(name, w=NFS):
            return ptmp.tile([C, w], F32, tag="pt", name=name)

        # ---- quantization level 1 (bit-exact round via comparisons) ----
        thr = sc["thr"]
        acc = pt("acc")
        nc.vector.tensor_scalar(acc[:], ush[:], float(thr[0]), None, ALU.is_ge)
        for j, T in enumerate(thr[1:]):
            nxt = pt(f"acc{j}")
            nc.vector.scalar_tensor_tensor(nxt[:], ush[:], float(T), acc[:],
                                           ALU.is_ge, ALU.add)
            acc = nxt
        v1 = pipe.tile([C, NFS], F32, tag="v1", name="v1")
        nc.vector.tensor_scalar(v1[:], acc[:], s1, sc["qmin_s1"],
                                ALU.mult, ALU.add)

        # ---- m = mean over members (seq sum, then single * fl(1/6)) ----
        msum = v1[:, 0:FS]
        for n in range(1, N):
            nxt = pt(f"msum{n}", FS)
            nc.vector.tensor_tensor(nxt[:], msum, v1[:, n * FS:(n + 1) * FS],
                                    ALU.add)
            msum = nxt[:]
        m = pipe.tile([C, FS], F32, tag="m", name="m")
        nc.scalar.mul(m[:], msum, sc["r6hi"])

        # ---- quantization level 2 (magic-number RNE round) ----
        d = pt("d")
        for n in range(N):
            nc.gpsimd.tensor_tensor(d[:, n * FS:(n + 1) * FS],
                                    ush[:, n * FS:(n + 1) * FS], m[:],
                                    ALU.subtract)
        phi = pt("phi")
        nc.vector.tensor_scalar(phi[:], d[:], sc["r2hi"], None, ALU.mult)
        t2 = pt("t2")
        nc.vector.scalar_tensor_tensor(t2[:], d[:], sc["r2lo"], phi[:],
                                       ALU.mult, ALU.add)
        t2m = pt("t2m")
        nc.scalar.activation(t2m[:], t2[:], AF.Copy, bias=MAGIC)
        q2f = pt("q2f")
        nc.scalar.activation(q2f[:], t2m[:], AF.Copy, bias=-MAGIC)
        v2 = pipe.tile([C, NFS], F32, tag="v2", name="v2")
        nc.vector.tensor_scalar(v2[:], q2f[:], s2, None, ALU.mult)

        # ---- gates (all 7 rows at once) ----
        RFS = (N + 1) * FS
        lnu = ptmp.tile([C, RFS], F32, tag="ptr", name="lnu")
        nc.scalar.activation(lnu[:], uu[:], AF.Ln)
        om = ptmp.tile([C, RFS], F32, tag="ptr", name="om")
        nc.vector.tensor_scalar(om[:], uu[:], -1.0, 1.0, ALU.mult, ALU.add)
        ln1m = ptmp.tile([C, RFS], F32, tag="ptr", name="ln1m")
        nc.scalar.activation(ln1m[:], om[:], AF.Ln)
        gg = ptmp.tile([C, RFS], F32, tag="ptr", name="gg")
        nc.vector.tensor_tensor(gg[:], lnu[:], ln1m[:], ALU.subtract)
        ge = ptmp.tile([C, RFS], F32, tag="ptr", name="ge")
        nc.gpsimd.tensor_tensor(ge[:], gg[:], ub[:], ALU.add)
        sg = ptmp.tile([C, RFS], F32, tag="ptr", name="sg")
        nc.scalar.activation(sg[:], ge[:], AF.Sigmoid)
        gr = ptmp.tile([C, RFS], F32, tag="ptr", name="gr")
        nc.vector.tensor_scalar(gr[:], sg[:], sc["sc_gate"], sc["zp2"],
                                ALU.mult, ALU.add)
        gate = pipe.tile([C, RFS], F32, tag="gate", name="gate")
        nc.vector.tensor_scalar(gate[:], gr[:], 0.0, 1.0, ALU.max, ALU.min)

        # ---- weights: w = v1*g1 + v2*g2*(g1>0) ----
        g2 = gate[:, N * FS:RFS]
        bpre = pt("bpre")
        for n in range(N):
            nc.gpsimd.tensor_tensor(bpre[:, n * FS:(n + 1) * FS],
                                    v2[:, n * FS:(n + 1) * FS], g2, ALU.mult)
        bsel = pt("bsel")
        nc.vector.scalar_tensor_tensor(bsel[:], gate[:, 0:NFS], 0.0, bpre[:],
                                       ALU.is_gt, ALU.mult)
        cc = pt("cc")
        nc.gpsimd.tensor_tensor(cc[:], v1[:], gate[:, 0:NFS], ALU.mult)
        wloc = pipe.tile([C, NFS], F32, tag="wloc", name="wloc")
        nc.vector.tensor_tensor(wloc[:], bsel[:], cc[:], ALU.add)

        # ---- stage -> AllGather -> readback ----
        # wstage[n, o, f] = wloc[o, n*FS+f]
        from concourse.tile_rust import add_dep_helper
        stage = nc.sync.dma_start(
            wstage_t[:, :, :].rearrange("n o f -> o n f"),
            wloc[:].rearrange("o (n f) -> o n f", n=N))
        ccop = nc.gpsimd.collective_compute(
            "AllGather", ALU.bypass,
            replica_groups=[list(range(N_CORES))],
            ins=[wstage_t[:]],
            outs=[wgath_t[:]],
        )
        add_dep_helper(ccop.ins, stage.ins, True, "stage before allgather")

        wt = [[wtp.tile([C, C], F32R, tag="wt", name=f"wt_{n}_{q}")
               for q in range(KQ)] for n in range(N)]

        for n in range(N):
            if n == 0:
                xsb = xsb0
            else:
                xsb = xpool.tile([C, XW], F32, tag="x", name=f"x_{n}")
                nc.sync.dma_start(xsb[:], xp_t[n])
            # fp32r matmul operands must be explicitly rounded by a compute op
            xr = xrpool.tile([C, XW], F32R, tag="xr", name=f"xr_{n}")
            nc.vector.tensor_copy(xr[:], xsb[:])

            # full member-n weights: gather i-blocks from all 8 cores
            wn = wnatp.tile([C, FD], F32, tag="wnat", name=f"wn_{n}")
            rb = nc.sync.dma_start(
                wn[:].rearrange("o (r f) -> o r f", r=N_CORES),
                wgath_t[:, n, :, :].rearrange("r o f -> o r f"))
            add_dep_helper(rb.ins, ccop.ins, True, "allgather before readback")

            wq = wn[:].rearrange("p (i q) -> p q i", q=KQ)
            for q in range(KQ):
                tp = ps_tp.tile([C, C], F32, tag="tp", name=f"tp_{n}_{q}")
                nc.tensor.transpose(tp[:], wq[:, q, :], ident[:])
                nc.any.tensor_copy(wt[n][q][:], tp[:])

            for off, cw in CHUNKS:
                ps = ps_cv.tile([C, 512], F32, tag="cv", name=f"cv_{n}_{off}")
                for ky in range(3):
                    for kx in range(3):
                        q = ky * 3 + kx
                        a = 1 + OUT0 + off + (ky - 1) * PW + (kx - 1)
                        nc.tensor.matmul(
                            ps[:, :cw],
                            wt[n][q][:],
                            xr[:, a:a + cw],
                            start=(q == 0), stop=(q == KQ - 1),
                        )
                ob = obufp.tile([C, 512], F32, tag="ob", name=f"ob_{n}_{off}")
                nc.any.tensor_copy(ob[:, :cw], ps[:, :cw])
                nc.sync.dma_start(out_t[n][:, off:off + cw], ob[:, :cw])

    _split_excess_dma_waits(nc)
    return nc


# ---------------------------------------------------------------------------
# entry point
# ---------------------------------------------------------------------------

_prog_cache = {}
last_results = None  # BassKernelResults of the most recent kernel() call

_AXON_SO = "/opt/axon/libaxon_pjrt.so"


def _build_ntff_hook():
    """(output_dir, device_ids) -> contextmanager driving NRT profiling via
    the axon PJRT .so — the slim-container equivalent of axon.trn.ntff_profile."""
    import contextlib
    import ctypes
    import sys as _sys

    if not os.path.exists(_AXON_SO):
        return None
    lib = ctypes.CDLL(_AXON_SO)
    if not hasattr(lib, "axon_start_nrt_profile"):
        return None
    lib.axon_start_nrt_profile.argtypes = [ctypes.POINTER(ctypes.c_int64),
                                           ctypes.c_size_t]
    lib.axon_start_nrt_profile.restype = ctypes.c_int64
    lib.axon_stop_nrt_profile.argtypes = [ctypes.c_char_p]
    lib.axon_stop_nrt_profile.restype = ctypes.c_int64

    @contextlib.contextmanager
    def _hook(output_dir, device_ids):
        import jax
        jax.devices()
        if device_ids:
            ids = (ctypes.c_int64 * len(device_ids))(*device_ids)
            rc = lib.axon_start_nrt_profile(ids, len(device_ids))
        else:
            rc = lib.axon_start_nrt_profile(None, 0)
        if rc != 0:
            raise RuntimeError(f"axon_start_nrt_profile rc={rc}")
        try:
            yield
        finally:
            n = lib.axon_stop_nrt_profile(str(output_dir).encode())
            print(f"profile: {n} file(s) written to {output_dir}",
                  file=_sys.stderr)

    return _hook


def _ensure_ntff_hook():
    """Make `antenv.axon_hooks.get_axon_ntff_profile_hook` importable so the
    BASS_TRACE path in run_bass_kernel_spmd works (or degrades gracefully)."""
    import sys as _sys
    import types

    try:
        from antenv.axon_hooks import get_axon_ntff_profile_hook  # noqa: F401
        return
    except ImportError:
        pass
    mod = types.ModuleType("antenv.axon_hooks")
    state = {}

    def get_axon_ntff_profile_hook():
        if "h" not in state:
            try:
                state["h"] = _build_ntff_hook()
            except Exception:
                state["h"] = None
        return state["h"]

    mod.get_axon_ntff_profile_hook = get_axon_ntff_profile_hook
    _sys.modules["antenv.axon_hooks"] = mod
    try:
        import antenv
        antenv.axon_hooks = mod
    except ImportError:
        pass


def _get_program(sc):
    key = (sc["s1"], sc["s2"], sc["qmin"], sc["thr"])
    if key not in _prog_cache:
        _prog_cache[key] = _build_program(sc)
    return _prog_cache[key]


def kernel(x, U, bp, u):
    x = np.ascontiguousarray(x, dtype=np.float32)
    U = np.ascontiguousarray(U, dtype=np.float32)
    bp = np.ascontiguousarray(bp, dtype=np.float32)
    u = np.ascontiguousarray(u, dtype=np.float32)
    B = x.shape[0]
    assert B == N_CORES * N and x.shape[1] == C

    sc = _host_scalars(U)
    nc = _get_program(sc)

    # zero-padded 66-wide spatial layout, one guard element in front
    grid = np.zeros((B, C, PW, PW), np.float32)
    grid[:, :, 1:H + 1, 1:W + 1] = x
    xp = np.zeros((B, C, XW), np.float32)
    xp[:, :, 1:1 + GRID] = grid.reshape(B, C, GRID)
    xp = xp.reshape(N_CORES, N, C, XW)

    # per-core weight-pipeline shards: core c gets in-channel block
    # [IB*c, IB*(c+1)) == free-dim slice [FS*c, FS*(c+1)) of each row
    U3 = U.reshape(N, C, FD)      # [n][o][i*9+q]
    bp3 = bp.reshape(N + 1, C, FD)
    u3 = u.reshape(N + 1, C, FD)
    ident = np.eye(C, dtype=np.float32)
    in_maps = []
    for c in range(N_CORES):
        sl = slice(FS * c, FS * (c + 1))
        ush = np.ascontiguousarray(
            U3[:, :, sl].transpose(1, 0, 2).reshape(C, N * FS))
        uuc = np.ascontiguousarray(
            u3[:, :, sl].transpose(1, 0, 2).reshape(C, (N + 1) * FS))
        ubc = np.ascontiguousarray(
            bp3[:, :, sl].transpose(1, 0, 2).reshape(C, (N + 1) * FS))
        in_maps.append({"xp": xp[c], "ush": ush, "uu": uuc, "ub": ubc,
                        "ident": ident})

    _ensure_ntff_hook()
    global last_results
    last_results = run_bass_kernel_spmd(nc, in_maps, list(range(N_CORES)))
    res = last_results.results

    out = np.empty((B, C, H, W), np.float32)
    for c in range(N_CORES):
        o = res[c]["out"].reshape(N, C, H, PW)
        out[c * N:(c + 1) * N] = o[:, :, :, 1:W + 1]
    return out


# revision 34
# speedup vs baseline: 1.2187x; 1.0595x over previous
"""Trainium2 Bass kernel for nn_Conv2d_shared_res_bit_baens.

Computes, for inputs x:(48,128,64,64) U:(6,147456) bp:(7,147456) u:(7,147456):
  - gumbel-sigmoid gates from (u, bp)
  - 2-level residual STE quantization of U  (round-to-nearest-even)
  - per-member 3x3 conv weights w[n] = v1*g1 + v2*g2*(g1>0)
  - grouped conv: out[b] = conv2d(x[b], W[b % 6], pad=1)

Sharding: data-parallel over batch across 8 NeuronCores, with the 48
(member, image) pairs assigned so each core needs at most TWO members'
weights: core c convolves 4 images of member A[c] and 2 of member B[c].
Each core then builds only its two weight sets (plus the full-D v1 pass that
the cross-member mean requires) — no collectives, and the weight pipeline
drops to ~1/3 of the replicated cost.  The conv itself is 9 shift-matmuls
(128x128 per tap, fp32r) accumulated in PSUM over a 66-wide zero-padded
spatial layout.
"""

import os
from contextlib import ExitStack

import numpy as np

import concourse.bass as bass
import concourse.mybir as mybir
import concourse.tile as tile
from concourse.bass_utils import run_bass_kernel_spmd

F32 = mybir.dt.float32
F32R = mybir.dt.float32r
AF = mybir.ActivationFunctionType
ALU = mybir.AluOpType

N_CORES = 8
N = 6                      # ensemble members
C = 128                    # in/out channels
D = 147456                 # flattened weight size per member
FD = D // C                # 1152 free-dim elements per partition
KQ = 9                     # 3x3 taps
H = W = 64
PW = 66                    # padded spatial width
GRID = PW * PW             # 4356
XW = 4360                  # x_pad row: [1 guard][4356 grid][3 tail guard]
OUT0 = PW                  # first output flat position (row y=1, x=0)
OUTW = 4224                # 64 rows * 66 cols
MAGIC = float(np.float32(12582912.0))   # 1.5 * 2**23 (RNE integer rounding)
ZP1, ZP2 = 1.2, -0.2
# conv output chunking: all chunks >=256 wide so fp32r matmuls run at 1 cyc/row
CHUNKS = [(0, 512), (512, 512), (1024, 512), (1536, 512), (2048, 512),
          (2560, 512), (3072, 512), (3584, 320), (3904, 320)]
assert sum(w for _, w in CHUNKS) == OUTW

# per-core member pair: 4 images of A[c], then 2 images of B[c]
MEM_A = [0, 0, 1, 1, 2, 3, 4, 5]
MEM_B = [2, 3, 4, 5, 3, 2, 4, 5]
N_A, N_B = 4, 2
IMGS = N_A + N_B           # images per core


def _assignment():
    """(member, g) pair for every (core, image-slot); g indexes the member's
    8 images (batch index b = 6*g + member)."""
    gctr = [0] * N
    assign = [[] for _ in range(N_CORES)]
    for c in range(N_CORES):
        for _ in range(N_A):
            m = MEM_A[c]
            assign[c].append((m, gctr[m]))
            gctr[m] += 1
    for c in range(N_CORES):
        for _ in range(N_B):
            m = MEM_B[c]
            assign[c].append((m, gctr[m]))
            gctr[m] += 1
    assert gctr == [8] * N
    return assign


ASSIGN = _assignment()


# ---------------------------------------------------------------------------
# host-side scalar prep (bit-exact mirrors of the fp32 scalar ops in the ref)
# ---------------------------------------------------------------------------

def _round_f32(x):
    return np.round(x.astype(np.float32)).astype(np.float32)


def _find_threshold(s1, k):
    """Smallest f32 v with round(fl(v / s1)) >= k  (s1 > 0, RNE round)."""
    pred = lambda v: float(np.round(np.float32(v) / s1)) >= k
    v = np.float32(s1 * np.float32(k - 0.5))
    step = np.float32(max(abs(float(v)), 1.0) * 1e-5)
    lo, hi = v, v
    while pred(lo):
        lo = np.float32(lo - step)
    while not pred(hi):
        hi = np.float32(hi + step)
    # bisect on the f32 bit lattice
    def bits(f):
        i = np.float32(f).view(np.int32).item()
        return i if i >= 0 else (0x80000000 - i)  # monotone map

    def unbits(i):
        i = i if i >= 0 else -(i - 0x80000000)
        if i >= 0x80000000:
            i = 0x80000000 - i
        return np.int32(i).view(np.float32)

    blo, bhi = bits(lo), bits(hi)
    while bhi - blo > 1:
        mid = (blo + bhi) // 2
        if pred(unbits(mid)):
            bhi = mid
        else:
            blo = mid
    return unbits(bhi)


def _host_scalars(U):
    beta = np.float32(U.max())
    alpha = np.float32(U.min())
    s1 = np.float32((beta - alpha) / np.float32(3.0))
    s2 = np.float32(s1 / np.float32(5.0))
    t = (U / s1).astype(np.float32)
    qr = _round_f32(t)
    qmin, qmax = int(qr.min()), int(qr.max())
    thr = [float(_find_threshold(s1, k)) for k in range(qmin + 1, qmax + 1)]
    # verify the comparison form reproduces round(U/s1) exactly
    qdev = np.full(U.shape, qmin, np.float32)
    for T in thr:
        qdev += (U >= np.float32(T)).astype(np.float32)
    assert np.array_equal(qdev, qr), "q1 threshold construction failed"
    # verify the device affine acc*s1 + (qmin*s1) hits fl(s1*q) for every q
    qmin_s1 = np.float32(np.float32(qmin) * s1)
    for q in range(qmin, qmax + 1):
        acc = np.float32(q - qmin)
        dev = np.float32(np.float32(acc * s1) + qmin_s1)
        ref = np.float32(s1 * np.float32(q))
        assert dev == ref, f"v1 affine inexact for q={q}"
    r2hi = np.float32(np.float64(1.0) / np.float64(s2))
    r2lo = np.float32(np.float64(1.0) / np.float64(s2) - np.float64(r2hi))
    sc_gate = np.float32(np.float32(ZP1) - np.float32(ZP2))
    return dict(
        s1=float(s1), s2=float(s2), qmin=qmin, thr=tuple(thr),
        qmin_s1=float(qmin_s1), r2hi=float(r2hi), r2lo=float(r2lo),
        sc_gate=float(sc_gate), zp2=float(np.float32(ZP2)),
        r6hi=float(np.float32(np.float64(1.0) / 6.0)),
    )


# ---------------------------------------------------------------------------
# post-pass: walrus allows only ONE sync wait per instruction
# ---------------------------------------------------------------------------

_NO_SPLIT_TYPES = ("InstNoOp", "InstEventSemaphore",
                   "InstBranch", "InstSemaphoreOp", "InstRegister")


def _split_excess_dma_waits(nc, max_waits=1):
    """walrus's pseudo-instruction formats support a single sync wait per
    instruction, but Tile can emit several (pool-slot reuse: reader WAR +
    prior-writer WAW + own DMA lane).  Move the excess onto NoOps on the
    issuing engine right before the instruction — the sequencer executes the
    waits in order, so semantics are unchanged."""
    import bass_rust

    func = nc.m.functions[0]
    for blk in func.blocks:
        insts = list(blk.instructions)
        new_insts = []
        changed = False
        for ins in insts:
            si = ins.sync_info
            tname = type(ins).__name__
            if (si is not None and len(si.on_wait) > max_waits
                    and not any(tname.startswith(p) for p in _NO_SPLIT_TYPES)):
                waits = list(si.on_wait)
                moved, kept = waits[:-max_waits], waits[-max_waits:]
                for j, w in enumerate(moved):
                    nop = mybir.InstNoOp(
                        name=f"{ins.name}-waitnop{j}", ins=[], outs=[])
                    nop.engine = ins.engine
                    nop.sync_info = bass_rust.SyncInfo(on_wait=[w],
                                                       on_update=[])
                    new_insts.append(nop)
                ins.sync_info = bass_rust.SyncInfo(
                    on_wait=kept, on_update=list(si.on_update))
                changed = True
            new_insts.append(ins)
        if changed:
            blk.instructions = new_insts


# ---------------------------------------------------------------------------
# device program
# ---------------------------------------------------------------------------

def _build_program(sc):
    nc = bass.Bass(target_bir_lowering=False, debug=False, trn_type="TRN2")

    # ufull[o, n*FD+f] = U[n, o*FD+f]  (member-major free dim) — for the mean
    uf_t = nc.dram_tensor("ufull", [C, N * FD], F32, kind="ExternalInput")
    # rows for this core's two members (A, B) and (for gates) the shared row
    uab_t = nc.dram_tensor("uab", [C, 2 * FD], F32, kind="ExternalInput")
    ug_t = nc.dram_tensor("ug", [C, 3 * FD], F32, kind="ExternalInput")
    bg_t = nc.dram_tensor("bg", [C, 3 * FD], F32, kind="ExternalInput")
    xp_t = nc.dram_tensor("xp", [IMGS, C, XW], F32, kind="ExternalInput")
    id_t = nc.dram_tensor("ident", [C, C], F32, kind="ExternalInput")
    out_t = nc.dram_tensor("out", [IMGS, C, OUTW], F32, kind="ExternalOutput")

    s1, s2 = sc["s1"], sc["s2"]

    with tile.TileContext(nc) as tc, ExitStack() as ctx:
        cpool = ctx.enter_context(tc.tile_pool(name="const", bufs=1))
        pipe = ctx.enter_context(tc.tile_pool(name="pipe", bufs=1))
        rowp = ctx.enter_context(tc.tile_pool(name="rows", bufs=2))
        ptmp = ctx.enter_context(tc.tile_pool(name="ptmp", bufs=6))
        wtp = ctx.enter_context(tc.tile_pool(name="wt", bufs=2 * KQ))
        xpool = ctx.enter_context(tc.tile_pool(name="xpool", bufs=1))
        xrpool = ctx.enter_context(tc.tile_pool(name="xrpool", bufs=2))
        obufp = ctx.enter_context(tc.tile_pool(name="obuf", bufs=4))
        ps_tp = ctx.enter_context(tc.tile_pool(name="pstp", bufs=2, space="PSUM"))
        ps_cv = ctx.enter_context(tc.tile_pool(name="pscv", bufs=6, space="PSUM"))

        ident = cpool.tile([C, C], F32)
        nc.sync.dma_start(ident[:], id_t[:])

        uab = pipe.tile([C, 2 * FD], F32, tag="uab", name="uab")
        nc.sync.dma_start(uab[:], uab_t[:])

        xsb0 = xpool.tile([C, XW], F32, tag="x", name="x_0")

        def pt(name):
            return ptmp.tile([C, FD], F32, tag="pt", name=name)

        thr = sc["thr"]
        HF = FD // 2

        def q1_acc(src_ap, tag):
            """acc = sum_k [src >= T_k] over a (C, FD) slice (DVE)."""
            acc = pt(f"acc_{tag}")
            nc.vector.tensor_scalar(acc[:], src_ap[:], float(thr[0]),
                                    None, ALU.is_ge)
            for j, T in enumerate(thr[1:]):
                nxt = pt(f"acc_{tag}{j}")
                nc.vector.scalar_tensor_tensor(nxt[:], src_ap[:], float(T),
                                               acc[:], ALU.is_ge, ALU.add)
                acc = nxt
            return acc

        # ---- full-D v1 pass (only needed for the cross-member mean) ----
        m = pipe.tile([C, FD], F32, tag="m", name="m")
        with tc.tile_pool(name="ph1", bufs=2) as p1:
            msum = None
            for n in range(N):
                un = p1.tile([C, FD], F32, tag="un", name=f"un_{n}",
                             bufs=3)
                nc.sync.dma_start(un[:], uf_t[:, n * FD:(n + 1) * FD])
                accn = q1_acc(un[:], f"f{n}")
                v1n = p1.tile([C, FD], F32, tag="v1n", name=f"v1n_{n}",
                              bufs=2)
                nc.scalar.activation(v1n[:], accn[:], AF.Copy,
                                     bias=sc["qmin_s1"], scale=s1)
                if msum is None:
                    msum = v1n
                else:
                    nxt = p1.tile([C, FD], F32, tag="msum",
                                  name=f"msum_{n}", bufs=2)
                    nc.vector.tensor_tensor(nxt[:, :HF], msum[:, :HF],
                                            v1n[:, :HF], ALU.add)
                    nc.gpsimd.tensor_tensor(nxt[:, HF:], msum[:, HF:],
                                            v1n[:, HF:], ALU.add)
                    msum = nxt
            nc.scalar.mul(m[:], msum[:], sc["r6hi"])

        # prefetch image 0 behind the phase-1 U reads
        nc.sync.dma_start(xsb0[:], xp_t[0])

        # ---- gates for rows A, B, shared (one row at a time) ----
        gate = pipe.tile([C, 3 * FD], F32, tag="gate", name="gate")
        for r in range(3):
            ur = rowp.tile([C, FD], F32, tag="ur", name=f"ur_{r}")
            nc.sync.dma_start(ur[:], ug_t[:, r * FD:(r + 1) * FD])
            br = rowp.tile([C, FD], F32, tag="br", name=f"br_{r}")
            nc.sync.dma_start(br[:], bg_t[:, r * FD:(r + 1) * FD])
            lnu = pt(f"lnu_{r}")
            nc.scalar.activation(lnu[:], ur[:], AF.Ln)
            om = pt(f"om_{r}")
            nc.scalar.activation(om[:], ur[:], AF.Copy, bias=1.0, scale=-1.0)
            ln1m = pt(f"ln1m_{r}")
            nc.scalar.activation(ln1m[:], om[:], AF.Ln)
            gg = pt(f"gg_{r}")
            nc.vector.tensor_tensor(gg[:], lnu[:], ln1m[:], ALU.subtract)
            ge = pt(f"ge_{r}")
            nc.gpsimd.tensor_tensor(ge[:], gg[:], br[:], ALU.add)
            sg = pt(f"sg_{r}")
            nc.scalar.activation(sg[:], ge[:], AF.Sigmoid)
            gr = pt(f"gr_{r}")
            nc.scalar.activation(gr[:], sg[:], AF.Copy, bias=sc["zp2"],
                                 scale=sc["sc_gate"])
            nc.vector.tensor_scalar(gate[:, r * FD:(r + 1) * FD], gr[:],
                                    0.0, 1.0, ALU.max, ALU.min)

        # ---- weights for the two members (per slot) ----
        g2 = gate[:, 2 * FD:3 * FD]
        wab = pipe.tile([C, 2 * FD], F32, tag="wab", name="wab")
        for s in range(2):
            us = uab[:, s * FD:(s + 1) * FD]
            g1 = gate[:, s * FD:(s + 1) * FD]
            accs = q1_acc(us, f"s{s}")
            v1s = pt(f"v1s_{s}")
            nc.scalar.activation(v1s[:], accs[:], AF.Copy,
                                 bias=sc["qmin_s1"], scale=s1)
            ds = pt(f"d_{s}")
            nc.gpsimd.tensor_tensor(ds[:], us, m[:], ALU.subtract)
            phis = pt(f"phi_{s}")
            nc.scalar.mul(phis[:], ds[:], sc["r2hi"])
            t2s = pt(f"t2_{s}")
            nc.vector.scalar_tensor_tensor(t2s[:], ds[:], sc["r2lo"],
                                           phis[:], ALU.mult, ALU.add)
            t2ms = pt(f"t2m_{s}")
            nc.scalar.activation(t2ms[:], t2s[:], AF.Copy, bias=MAGIC)
            q2fs = pt(f"q2f_{s}")
            nc.scalar.activation(q2fs[:], t2ms[:], AF.Copy, bias=-MAGIC)
            v2s = pt(f"v2_{s}")
            nc.scalar.mul(v2s[:], q2fs[:], s2)
            bpre = pt(f"bpre_{s}")
            nc.gpsimd.tensor_tensor(bpre[:], v2s[:], g2, ALU.mult)
            bsel = pt(f"bsel_{s}")
            nc.vector.scalar_tensor_tensor(bsel[:], g1, 0.0, bpre[:],
                                           ALU.is_gt, ALU.mult)
            ccs = pt(f"cc_{s}")
            nc.gpsimd.tensor_tensor(ccs[:], v1s[:], g1, ALU.mult)
            nc.vector.tensor_tensor(wab[:, s * FD:(s + 1) * FD], bsel[:],
                                    ccs[:], ALU.add)

        # ---- transposes: per slot s, per tap q: (o,i) -> (i,o) ----
        wt = [[wtp.tile([C, C], F32R, tag="wt", name=f"wt_{s}_{q}")
               for q in range(KQ)] for s in range(2)]
        wq = wab[:].rearrange("p (s i q) -> p s q i", s=2, q=KQ)
        for s in range(2):
            for q in range(KQ):
                tp = ps_tp.tile([C, C], F32, tag="tp", name=f"tp_{s}_{q}")
                nc.tensor.transpose(tp[:], wq[:, s, q, :], ident[:])
                nc.any.tensor_copy(wt[s][q][:], tp[:])

        # ---- conv: 4 images of slot 0, then 2 of slot 1 ----
        for j in range(IMGS):
            slot = 0 if j < N_A else 1
            if j == 0:
                xsb = xsb0
            else:
                xsb = xpool.tile([C, XW], F32, tag="x", name=f"x_{j}")
                nc.sync.dma_start(xsb[:], xp_t[j])
            # fp32r matmul operands must be explicitly rounded by a compute op
            xr = xrpool.tile([C, XW], F32R, tag="xr", name=f"xr_{j}")
            nc.vector.tensor_copy(xr[:], xsb[:])

            for off, cw in CHUNKS:
                ps = ps_cv.tile([C, 512], F32, tag="cv", name=f"cv_{j}_{off}")
                for ky in range(3):
                    for kx in range(3):
                        q = ky * 3 + kx
                        a = 1 + OUT0 + off + (ky - 1) * PW + (kx - 1)
                        nc.tensor.matmul(
                            ps[:, :cw],
                            wt[slot][q][:],
                            xr[:, a:a + cw],
                            start=(q == 0), stop=(q == KQ - 1),
                        )
                ob = obufp.tile([C, 512], F32, tag="ob", name=f"ob_{j}_{off}")
                nc.any.tensor_copy(ob[:, :cw], ps[:, :cw])
                nc.sync.dma_start(out_t[j][:, off:off + cw], ob[:, :cw])

    _split_excess_dma_waits(nc)
    return nc


# ---------------------------------------------------------------------------
# entry point
# ---------------------------------------------------------------------------

_prog_cache = {}
last_results = None  # BassKernelResults of the most recent kernel() call

_AXON_SO = "/opt/axon/libaxon_pjrt.so"


def _build_ntff_hook():
    """(output_dir, device_ids) -> contextmanager driving NRT profiling via
    the axon PJRT .so — the slim-container equivalent of axon.trn.ntff_profile."""
    import contextlib
    import ctypes
    import sys as _sys

    if not os.path.exists(_AXON_SO):
        return None
    lib = ctypes.CDLL(_AXON_SO)
    if not hasattr(lib, "axon_start_nrt_profile"):
        return None
    lib.axon_start_nrt_profile.argtypes = [ctypes.POINTER(ctypes.c_int64),
                                           ctypes.c_size_t]
    lib.axon_start_nrt_profile.restype = ctypes.c_int64
    lib.axon_stop_nrt_profile.argtypes = [ctypes.c_char_p]
    lib.axon_stop_nrt_profile.restype = ctypes.c_int64

    @contextlib.contextmanager
    def _hook(output_dir, device_ids):
        import jax
        jax.devices()
        if device_ids:
            ids = (ctypes.c_int64 * len(device_ids))(*device_ids)
            rc = lib.axon_start_nrt_profile(ids, len(device_ids))
        else:
            rc = lib.axon_start_nrt_profile(None, 0)
        if rc != 0:
            raise RuntimeError(f"axon_start_nrt_profile rc={rc}")
        try:
            yield
        finally:
            n = lib.axon_stop_nrt_profile(str(output_dir).encode())
            print(f"profile: {n} file(s) written to {output_dir}",
                  file=_sys.stderr)

    return _hook


def _ensure_ntff_hook():
    """Make `antenv.axon_hooks.get_axon_ntff_profile_hook` importable so the
    BASS_TRACE path in run_bass_kernel_spmd works (or degrades gracefully)."""
    import sys as _sys
    import types

    try:
        from antenv.axon_hooks import get_axon_ntff_profile_hook  # noqa: F401
        return
    except ImportError:
        pass
    mod = types.ModuleType("antenv.axon_hooks")
    state = {}

    def get_axon_ntff_profile_hook():
        if "h" not in state:
            try:
                state["h"] = _build_ntff_hook()
            except Exception:
                state["h"] = None
        return state["h"]

    mod.get_axon_ntff_profile_hook = get_axon_ntff_profile_hook
    _sys.modules["antenv.axon_hooks"] = mod
    try:
        import antenv
        antenv.axon_hooks = mod
    except ImportError:
        pass


def _get_program(sc):
    key = (sc["s1"], sc["s2"], sc["qmin"], sc["thr"])
    if key not in _prog_cache:
        _prog_cache[key] = _build_program(sc)
    return _prog_cache[key]


def kernel(x, U, bp, u):
    x = np.ascontiguousarray(x, dtype=np.float32)
    U = np.ascontiguousarray(U, dtype=np.float32)
    bp = np.ascontiguousarray(bp, dtype=np.float32)
    u = np.ascontiguousarray(u, dtype=np.float32)
    B = x.shape[0]
    assert B == N_CORES * N and x.shape[1] == C

    sc = _host_scalars(U)
    nc = _get_program(sc)

    # zero-padded 66-wide spatial layout, one guard element in front
    grid = np.zeros((B, C, PW, PW), np.float32)
    grid[:, :, 1:H + 1, 1:W + 1] = x
    xpad = np.zeros((B, C, XW), np.float32)
    xpad[:, :, 1:1 + GRID] = grid.reshape(B, C, GRID)

    U3 = U.reshape(N, C, FD)
    bp3 = bp.reshape(N + 1, C, FD)
    u3 = u.reshape(N + 1, C, FD)
    ufull = np.ascontiguousarray(
        U3.transpose(1, 0, 2).reshape(C, N * FD))
    ident = np.eye(C, dtype=np.float32)

    in_maps = []
    for c in range(N_CORES):
        a_m, b_m = MEM_A[c], MEM_B[c]
        uab = np.ascontiguousarray(
            U3[[a_m, b_m]].transpose(1, 0, 2).reshape(C, 2 * FD))
        ug = np.ascontiguousarray(
            u3[[a_m, b_m, N]].transpose(1, 0, 2).reshape(C, 3 * FD))
        bg = np.ascontiguousarray(
            bp3[[a_m, b_m, N]].transpose(1, 0, 2).reshape(C, 3 * FD))
        xp = np.stack([xpad[6 * g + m] for (m, g) in ASSIGN[c]])
        in_maps.append({"ufull": ufull, "uab": uab, "ug": ug, "bg": bg,
                        "xp": xp, "ident": ident})

    _ensure_ntff_hook()
    global last_results
    last_results = run_bass_kernel_spmd(nc, in_maps, list(range(N_CORES)))
    res = last_results.results

    out = np.empty((B, C, H, W), np.float32)
    for c in range(N_CORES):
        o = res[c]["out"].reshape(IMGS, C, H, PW)
        for j, (m, g) in enumerate(ASSIGN[c]):
            out[6 * g + m] = o[j, :, :, 1:W + 1]
    return out
